# revision 40
# baseline (speedup 1.0000x reference)
import os
import sys
import types

import numpy as np

sys.path.insert(0, "/opt/trn_rl_repo")

import ml_dtypes  # noqa: E402
import concourse.mybir as mybir  # noqa: E402
import concourse.tile as tile  # noqa: E402
from concourse import bacc  # noqa: E402
from concourse.bass import ds, ts  # noqa: E402
from concourse.bass_utils import run_bass_kernel_spmd  # noqa: E402

BF16 = mybir.dt.bfloat16
F32 = mybir.dt.float32
bfdt = ml_dtypes.bfloat16
AF = mybir.ActivationFunctionType
ALU = mybir.AluOpType

B, D, N = 4, 512, 2048
H, KVH, DH = 8, 2, 64
CONTEXT_LEN = 4096
NLOC = 1024  # tokens per core
P = 128
NCORES = 8
HP = H // 2  # head pairs
NCH = N // P  # 16 key chunks of 128
NCH_LOC = NLOC // P  # 8 local key chunks

_CACHE = {}


def _enable_trace_hook():
    """Register the NTFF profile hook (missing antenv.axon_hooks shim)."""
    try:
        import antenv

        if "antenv.axon_hooks" in sys.modules:
            return
        mod = types.ModuleType("antenv.axon_hooks")

        def set_axon_ntff_profile_hook(h):
            mod._hook = h

        def get_axon_ntff_profile_hook():
            return getattr(mod, "_hook", None)

        mod.set_axon_ntff_profile_hook = set_axon_ntff_profile_hook
        mod.get_axon_ntff_profile_hook = get_axon_ntff_profile_hook
        sys.modules["antenv.axon_hooks"] = mod
        antenv.axon_hooks = mod
        from trn_agent_boot.trn_boot import _ntff_profile_via_ctypes

        set_axon_ntff_profile_hook(_ntff_profile_via_ctypes("/opt/axon/libaxon_pjrt.so"))
    except Exception:
        pass


def _build(TRIVIAL_GB, TRIVIAL_BO):
    nc = bacc.Bacc(None, target_bir_lowering=False, debug=False)
    dp = nc.declare_dram_parameter

    x_e = dp("x", [4, P, NLOC], F32, isOutput=False)
    wq_e = dp("wq", [P, 4, 512], BF16, isOutput=False)
    wqs_e = dp("wqs", [P, 4, 512], BF16, isOutput=False)
    wk_e = dp("wk", [P, 4, 128], BF16, isOutput=False)
    wks_e = dp("wks", [P, 4, 128], BF16, isOutput=False)
    wv_e = dp("wv", [P, 4, 128], BF16, isOutput=False)
    wo_e = dp("wo", [P, 4, 512], BF16, isOutput=False)
    cq_e = dp("cq", [P, NLOC], F32, isOutput=False)
    sq_e = dp("sq", [P, NLOC], F32, isOutput=False)
    ck_e = dp("ck", [P, NLOC], F32, isOutput=False)
    sk_e = dp("sk", [P, NLOC], F32, isOutput=False)
    gam_e = dp("gam", [P, 4], F32, isOutput=False)  # gamma per (p, chunk)
    bet_e = dp("bet", [P, 4], F32, isOutput=False)  # beta per (p, chunk)
    bo_e = dp("bo", [P, 4], F32, isOutput=False)  # bout per (p, chunk)
    ones_e = dp("ones", [P, 1], F32, isOutput=False)
    onesb_e = dp("onesb", [P, 1], BF16, isOutput=False)
    out_e = dp("out", [4, P, NLOC], F32, isOutput=True)

    with tile.TileContext(nc) as tc:
        with (
            tc.tile_pool(name="persist", bufs=1) as PS,
            tc.tile_pool(name="tmp", bufs=2) as TMP,
            tc.tile_pool(name="tmp4", bufs=4) as TMP4,
            tc.tile_pool(name="exp", bufs=3) as EXPP,
            tc.tile_pool(name="dram", bufs=1, space="DRAM") as DRAM,
        ):
            # ---------------- phase A: inputs -> SBUF ----------------
            SQP_cm = tc.tile_pool(name="sq_pool", bufs=1)
            SQP = SQP_cm.__enter__()
            x_sb = [
                [SQP.tile([P, 512], F32, name=f"x{c}_{tq}") for tq in range(2)]
                for c in range(4)
            ]
            for c in range(4):
                for tq in range(2):
                    nc.sync.dma_start(x_sb[c][tq][:], x_e[c][:, ts(tq, 512)])
            ones_sb = PS.tile([P, 1], F32, name="ones")
            nc.sync.dma_start(ones_sb[:], ones_e[:])
            onesb_sb = PS.tile([P, 1], BF16, name="onesb")
            nc.sync.dma_start(onesb_sb[:], onesb_e[:])
            gam_sb = PS.tile([P, 4], F32, name="gam")
            nc.sync.dma_start(gam_sb[:], gam_e[:])
            bet_sb = PS.tile([P, 4], F32, name="bet")
            nc.sync.dma_start(bet_sb[:], bet_e[:])
            bo_sb = PS.tile([P, 4], F32, name="bo")
            nc.sync.dma_start(bo_sb[:], bo_e[:])
            wk_sb = PS.tile([P, 4, 128], BF16, name="wk")
            nc.sync.dma_start(wk_sb[:], wk_e[:])
            wks_sb = PS.tile([P, 4, 128], BF16, name="wks")
            nc.sync.dma_start(wks_sb[:], wks_e[:])
            wv_sb = PS.tile([P, 4, 128], BF16, name="wv")
            nc.sync.dma_start(wv_sb[:], wv_e[:])
            ck_sb = PS.tile([P, NLOC], F32, name="ck")
            nc.sync.dma_start(ck_sb[:], ck_e[:])
            sk_sb = PS.tile([P, NLOC], F32, name="sk")
            nc.sync.dma_start(sk_sb[:], sk_e[:])
            wq_sb = PS.tile([P, 4, 512], BF16, name="wq")
            nc.sync.dma_start(wq_sb[:], wq_e[:])
            wqs_sb = PS.tile([P, 4, 512], BF16, name="wqs")
            nc.sync.dma_start(wqs_sb[:], wqs_e[:])
            cq_sb = PS.tile([P, NLOC], F32, name="cq")
            nc.sync.dma_start(cq_sb[:], cq_e[:])
            sq_sb = PS.tile([P, NLOC], F32, name="sq")
            nc.sync.dma_start(sq_sb[:], sq_e[:])
            wo_sb = PS.tile([P, 4, 512], BF16, name="wo")
            nc.sync.dma_start(wo_sb[:], wo_e[:])

            # v lhsT store: slot s = 2*chunk + kvh, [tk, dh+1] with ones col
            v_loc = PS.tile([P, 2 * NCH_LOC, DH + 1], BF16, name="vloc")
            nc.gpsimd.memset(v_loc[:, :, DH : DH + 1], 1.0)
            v_rem = [
                PS.tile([P, NCH_LOC, DH + 1], BF16, name=f"vrem{h}") for h in range(2)
            ]
            for h in range(2):
                nc.gpsimd.memset(v_rem[h][:, :, DH : DH + 1], 1.0)

            xnb = [PS.tile([P, NLOC], BF16, name=f"xnb{c}") for c in range(4)]
            qr_sb = [PS.tile([P, NLOC], BF16, name=f"qr{i}") for i in range(HP)]
            k_bf = PS.tile([P, NLOC], BF16, name="kbf")
            k_rem = [PS.tile([P, 512], BF16, name=f"krem{h}") for h in range(2)]
            vcp_sb = PS.tile([P, NCH_LOC, 128], BF16, name="vcp")
            ohat = [PS.tile([P, NLOC], BF16, name=f"oh{i}") for i in range(HP)]

            ag_in = DRAM.tile([2, P, NLOC], BF16)
            ag_out = DRAM.tile([2, 2, P, NLOC], BF16)

            # ---------------- phase B: layernorm ----------------
            with tc.tile_pool(name="ps_b1", bufs=1, space="PSUM") as PB1:
                stats = PB1.tile([1, 4, 512], F32, name="stats")
                xsq = [
                    [SQP.tile([P, 512], BF16, name=f"xsq{c}_{tq}") for tq in range(2)]
                    for c in range(4)
                ]
                for c in range(4):
                    for h2 in range(2):
                        nc.scalar.activation(
                            xsq[c][h2][:], x_sb[c][h2][:], AF.Square
                        )
                for tq in range(2):
                    for c in range(4):
                        nc.tensor.matmul(
                            stats[:, tq, :], ones_sb[:], x_sb[c][tq][:],
                            start=(c == 0), stop=(c == 3),
                        )
                for tq in range(2):
                    for c in range(4):
                        nc.tensor.matmul(
                            stats[:, 2 + tq, :], onesb_sb[:], xsq[c][tq][:],
                            start=(c == 0), stop=(c == 3),
                        )
                mu_sb = TMP4.tile([1, NLOC], F32, tag="ln")
                ex2_sb = TMP4.tile([1, NLOC], F32, tag="ln")
                musq_sb = TMP4.tile([1, NLOC], F32, tag="ln")
                var_sb = TMP4.tile([1, NLOC], F32, tag="ln")
                nc.scalar.mul(mu_sb[:], stats[:, 0:2, :].rearrange("p a b -> p (a b)"), 1.0 / 512)
                nc.scalar.activation(
                    ex2_sb[:], stats[:, 2:4, :].rearrange("p a b -> p (a b)"),
                    AF.Copy, bias=1e-5, scale=1.0 / 512,
                )
                nc.vector.tensor_mul(musq_sb[:], mu_sb[:], mu_sb[:])
                nc.vector.tensor_tensor(var_sb[:], ex2_sb[:], musq_sb[:], ALU.subtract)
                # rstd = exp(-0.5 * ln(var)) -- Ln and Exp share a table set
                sd_sb = TMP4.tile([1, NLOC], F32, tag="ln")
                rstd_sb = TMP4.tile([1, NLOC], F32, tag="ln")
                nc.scalar.activation(sd_sb[:], var_sb[:], AF.Ln)
                nc.scalar.activation(rstd_sb[:], sd_sb[:], AF.Exp, scale=-0.5)
                # rstd and mu*rstd broadcast to all 128 partitions
                rstd_bc = SQP.tile([P, NLOC], F32, name="rstdbc")
                nc.gpsimd.partition_broadcast(rstd_bc[:], rstd_sb[0:1, :])
                mrs_sb = TMP4.tile([1, NLOC], F32, tag="ln")
                nc.vector.tensor_mul(mrs_sb[:], mu_sb[:], rstd_sb[:])
                mrs_bc = SQP.tile([P, NLOC], F32, name="mrsbc")
                nc.gpsimd.partition_broadcast(mrs_bc[:], mrs_sb[0:1, :])

            # xn = ((x * rstd) - mu*rstd) [* gamma + beta]   (bf16 out)
            for c in range(4):
                for tq in range(2):
                    t1 = TMP.tile([P, 512], F32, tag="th")
                    nc.vector.tensor_mul(
                        t1[:], x_sb[c][tq][:], rstd_bc[:, ts(tq, 512)]
                    )
                    if TRIVIAL_GB:
                        nc.vector.tensor_tensor(
                            xnb[c][:, ts(tq, 512)], t1[:],
                            mrs_bc[:, ts(tq, 512)], ALU.subtract,
                        )
                    else:
                        t2 = TMP.tile([P, 512], F32, tag="th")
                        nc.vector.tensor_tensor(
                            t2[:], t1[:], mrs_bc[:, ts(tq, 512)], ALU.subtract
                        )
                        nc.vector.tensor_scalar(
                            xnb[c][:, ts(tq, 512)], t2[:],
                            gam_sb[:, c : c + 1], bet_sb[:, c : c + 1],
                            ALU.mult, ALU.add,
                        )
            SQP_cm.__exit__(None, None, None)

            # ---------------- phase C1: k/v projection, rotary, allgather ----------------
            with tc.tile_pool(name="ps_c1", bufs=1, space="PSUM") as PC:
                kp0 = PC.tile([P, 2, 512], F32, name="kp0")
                kp1 = PC.tile([P, 2, 512], F32, name="kp1")
                for sw, (kps, w) in enumerate(((kp0, wk_sb), (kp1, wks_sb))):
                    for tq in range(2):
                        for c in range(4):
                            nc.tensor.matmul(
                                kps[:, tq, :], w[:, c, :], xnb[c][:, ts(tq, 512)],
                                start=(c == 0), stop=(c == 3),
                            )
                t1 = TMP.tile([P, NLOC], F32, tag="t")
                t2 = TMP.tile([P, NLOC], F32, tag="t")
                nc.vector.tensor_mul(t1[:], ck_sb[:], kp0[:].rearrange("p a b -> p (a b)"))
                nc.vector.tensor_mul(t2[:], sk_sb[:], kp1[:].rearrange("p a b -> p (a b)"))
                nc.vector.tensor_add(k_bf[:], t1[:], t2[:])

                for c8 in range(NCH_LOC):
                    vp = PC.tile([P, 128], F32, name=f"vp{c8 % 2}")
                    for c in range(4):
                        nc.tensor.matmul(
                            vp[:], xnb[c][:, ts(c8, 128)], wv_sb[:, c, :],
                            start=(c == 0), stop=(c == 3),
                        )
                    nc.vector.tensor_copy(vcp_sb[:, c8, :], vp[:])

                nc.sync.dma_start(ag_in[0], k_bf[:])
                nc.sync.dma_start(
                    ag_in[1], vcp_sb[:].rearrange("p a b -> p (a b)")
                )
                nc.gpsimd.collective_compute(
                    "AllGather",
                    ALU.bypass,
                    ins=[ag_in[:]],
                    outs=[ag_out[:]],
                    replica_groups=[[0, 1], [2, 3], [4, 5], [6, 7]],
                )

            # local v -> slots 0..15
            nc.vector.tensor_copy(
                v_loc[:, :, 0:DH],
                vcp_sb[:].rearrange("p a (g d) -> p (a g) d", g=2),
            )

            # ---------------- phase D: attention main loop ----------------
            # Block (hp, tq) halves. The first 4 blocks run their LOCAL key
            # half first (AV partial spilled to SBUF) so the AllGather is
            # hidden; remotes resume once it lands. Per group: scores
            # matmuls, then AV of the previous group (exp already done), then
            # exp -- the in-order PE never waits on the ACT.
            spills = {}

            def emit_qproj(PAV, i):
                qc = TMP4.tile([P, NLOC], BF16, tag="qcs")
                qs = TMP4.tile([P, NLOC], BF16, tag="qcs")
                for tq in range(2):
                    qq_ps = PAV.tile([P, 512], F32, tag="avA", name="qq_ps")
                    qs_ps = PAV.tile([P, 512], F32, tag="avB", name="qs_ps")
                    for kps, w in ((qq_ps, wq_sb), (qs_ps, wqs_sb)):
                        for c in range(4):
                            nc.tensor.matmul(
                                kps[:], w[:, c, ts(i, 128)],
                                xnb[c][:, ts(tq, 512)],
                                start=(c == 0), stop=(c == 3),
                            )
                    nc.vector.tensor_copy(qc[:, ts(tq, 512)], qq_ps[:])
                    nc.vector.tensor_copy(qs[:, ts(tq, 512)], qs_ps[:])
                t1 = TMP.tile([P, NLOC], F32, tag="t")
                t2 = TMP.tile([P, NLOC], F32, tag="t")
                nc.vector.tensor_mul(t1[:], cq_sb[:], qc[:])
                nc.vector.tensor_mul(t2[:], sq_sb[:], qs[:])
                nc.vector.tensor_add(qr_sb[i][:], t1[:], t2[:])

            def emit_epilogue(hp, tq, oA, oB, restore):
                # copy/merge out of PSUM first (frees the AV bank), then
                # divide by the denominator row and write o_hat
                sA = TMP.tile([DH + 1, 512], F32, tag="sum")
                sB = TMP.tile([DH + 1, 512], F32, tag="sum")
                if restore:
                    cpA, cpB = spills[(hp, tq)]
                    nc.vector.tensor_add(sA[:], oA[:], cpA[:])
                    nc.vector.tensor_add(sB[:], oB[:], cpB[:])
                else:
                    nc.vector.tensor_copy(sA[:], oA[:])
                    nc.vector.tensor_copy(sB[:], oB[:])
                den2 = TMP.tile([1, 1024], F32, tag="den")
                nc.vector.tensor_copy(den2[0:1, 0:512], sA[DH : DH + 1, :])
                nc.vector.tensor_copy(den2[0:1, 512:1024], sB[DH : DH + 1, :])
                rec2 = TMP.tile([1, 1024], F32, tag="den")
                nc.vector.reciprocal_approx_fast(rec2[:], den2[:])
                pbA = TMP.tile([64, 512], F32, tag="pb")
                pbB = TMP.tile([64, 512], F32, tag="pb")
                nc.gpsimd.partition_broadcast(pbA[:], rec2[0:1, 0:512])
                nc.gpsimd.partition_broadcast(pbB[:], rec2[0:1, 512:1024])
                nc.vector.tensor_mul(ohat[hp][0:64, ts(tq, 512)], sA[0:DH, :], pbA[:])
                nc.vector.tensor_mul(ohat[hp][64:128, ts(tq, 512)], sB[0:DH, :], pbB[:])

            def emit_spill(hp, tq, oA, oB):
                cpA = PS.tile([DH + 1, 512], F32, name=f"spA{hp}{tq}")
                cpB = PS.tile([DH + 1, 512], F32, name=f"spB{hp}{tq}")
                nc.vector.tensor_copy(cpA[:], oA[:])
                nc.vector.tensor_copy(cpB[:], oB[:])
                spills[(hp, tq)] = (cpA, cpB)

            def run_stream(PSC, PAV, plan):
                """plan: list of (hp, tq, chunks, mode) where mode is
                'spill' or 'epi' or 'epi_restore'. One flat software
                pipeline across all blocks: scores(i) | AV(i-1) | exp(i)."""
                stream = []
                for hp, tq, chunks, mode in plan:
                    slots = [(par, c) for c in chunks for par in range(2)]
                    grps = [slots[i : i + 3] for i in range(0, len(slots), 3)]
                    for gi, g in enumerate(grps):
                        stream.append(
                            (hp, tq, g, chunks[0], chunks[-1],
                             gi == 0, gi == len(grps) - 1, mode)
                        )
                prev = None
                avt = {}
                for it in stream + [None]:
                    if it is not None:
                        hp, tq, grp, _, _, _, _, _ = it
                        sc = PSC.tile([P, 3, 512], F32, tag="sc")
                        for pos, (par, c) in enumerate(grp):
                            if c < NCH_LOC:
                                ksrc = k_bf[:, ts(c, 128)]
                            else:
                                cr = c - NCH_LOC
                                ksrc = k_rem[cr // 4][:, ts(cr % 4, 128)]
                            nc.tensor.matmul(
                                sc[:, pos, :],
                                ksrc[64 * par : 64 * (par + 1), :],
                                qr_sb[hp][64 * par : 64 * (par + 1), ts(tq, 512)],
                                start=True, stop=True,
                                tile_position=(64 * par, 0),
                            )
                    if prev is not None:
                        ep, (php, ptq, pgrp, c0, c1, pfirst, plast, pmode) = prev
                        if pfirst:
                            av_a = PAV.tile([DH + 1, 512], F32, tag="avA", name="av_a")
                            av_b = PAV.tile([DH + 1, 512], F32, tag="avB", name="av_b")
                            avt[(php, ptq)] = (av_a, av_b)
                        oA, oB = avt[(php, ptq)]
                        for pos, (par, c) in enumerate(pgrp):
                            if c < NCH_LOC:
                                vt = v_loc[:, 2 * c + par, :]
                            else:
                                cr = c - NCH_LOC
                                vt = v_rem[cr // 4][:, 2 * (cr % 4) + par, :]
                            nc.tensor.matmul(
                                oA[:] if par == 0 else oB[:],
                                vt,
                                ep[:, ts(pos, 512)],
                                start=(c == c0), stop=(c == c1),
                            )
                        if plast:
                            oA, oB = avt.pop((php, ptq))
                            if pmode == "spill":
                                emit_spill(php, ptq, oA, oB)
                            else:
                                emit_epilogue(php, ptq, oA, oB, pmode == "epi_restore")
                    if it is not None:
                        e = EXPP.tile([P, 1536], BF16, tag="e")
                        nc.scalar.activation(
                            e[:, 0 : 512 * len(it[2])],
                            sc[:, 0 : len(it[2]), :].rearrange("p a b -> p (a b)"),
                            AF.Exp, scale=0.125,
                        )
                        prev = (e, it)

            LOC = list(range(NCH_LOC))
            REM = list(range(NCH_LOC, NCH))
            with (
                tc.tile_pool(name="ps_sc", bufs=2, space="PSUM") as PSC,
                tc.tile_pool(name="ps_av", bufs=1, space="PSUM") as PAV,
            ):
                emit_qproj(PAV, 0)
                run_stream(PSC, PAV, [(0, 0, LOC, "spill"), (0, 1, LOC, "spill")])
                emit_qproj(PAV, 1)
                run_stream(PSC, PAV, [(1, 0, LOC, "spill"), (1, 1, LOC, "spill")])
                emit_qproj(PAV, 2)
                emit_qproj(PAV, 3)

                # remote kv recovery: remote = (ag0 + ag1) - local  (exact),
                # done per token-half so remote scores start ASAP
                for h in range(2):
                    agk0 = TMP.tile([P, 512], BF16, tag="ag")
                    agk1 = TMP.tile([P, 512], BF16, tag="ag")
                    nc.sync.dma_start(agk0[:], ag_out[0, 0][:, ts(h, 512)])
                    nc.sync.dma_start(agk1[:], ag_out[1, 0][:, ts(h, 512)])
                    tk = TMP.tile([P, 512], F32, tag="th")
                    nc.vector.tensor_add(tk[:], agk0[:], agk1[:])
                    nc.vector.tensor_tensor(
                        k_rem[h][:], tk[:], k_bf[:, ts(h, 512)], ALU.subtract
                    )
                for h in range(2):
                    agv0 = TMP.tile([P, 512], BF16, tag="ag")
                    agv1 = TMP.tile([P, 512], BF16, tag="ag")
                    nc.sync.dma_start(agv0[:], ag_out[0, 1][:, ts(h, 512)])
                    nc.sync.dma_start(agv1[:], ag_out[1, 1][:, ts(h, 512)])
                    tv = TMP.tile([P, 512], F32, tag="th")
                    nc.vector.tensor_add(tv[:], agv0[:], agv1[:])
                    nc.vector.tensor_tensor(
                        v_rem[h][:, :, 0:DH],
                        tv[:].rearrange("p (a g d) -> p (a g) d", g=2, d=DH),
                        vcp_sb[:, ts(h, 4), :].rearrange("p a (g d) -> p (a g) d", g=2),
                        ALU.subtract,
                    )

                run_stream(PSC, PAV, [
                    (2, 0, LOC + REM, "epi"), (2, 1, LOC + REM, "epi"),
                    (3, 0, LOC + REM, "epi"), (3, 1, LOC + REM, "epi"),
                    (0, 0, REM, "epi_restore"), (0, 1, REM, "epi_restore"),
                    (1, 0, REM, "epi_restore"), (1, 1, REM, "epi_restore"),
                ])

            # ---------------- phase E: output projection + residual ----------------
            with tc.tile_pool(name="ps_e", bufs=4, space="PSUM") as PE_:
                for mc in range(4):
                    for tq in range(2):
                        yps = PE_.tile([P, 512], F32, tag="yps")
                        for kc in range(4):
                            nc.tensor.matmul(
                                yps[:], wo_sb[:, kc, ts(mc, 128)],
                                ohat[kc][:, ts(tq, 512)],
                                start=(kc == 0), stop=(kc == 3),
                            )
                        yt = TMP.tile([P, 512], F32, tag="yout")
                        nc.vector.tensor_add(yt[:], yps[:], xnb[mc][:, ts(tq, 512)])
                        if TRIVIAL_BO:
                            yo = yt
                        else:
                            yo = TMP.tile([P, 512], F32, tag="yout")
                            nc.vector.tensor_scalar_add(
                                yo[:], yt[:], bo_sb[:, mc : mc + 1]
                            )
                        for dq in range(2):
                            nc.sync.dma_start(
                                out_e[mc, :, ds(tq * 512 + dq * 256, 256)],
                                yo[:, ts(dq, 256)],
                            )

    nc.compile()
    return nc


def _host_inputs(x, gamma, beta, Wq, Wkv, Wout, bout):
    """Build the 8 per-core input maps."""
    x = np.asarray(x, np.float32)
    gamma = np.asarray(gamma, np.float32)
    beta = np.asarray(beta, np.float32)
    Wq = np.asarray(Wq, np.float32)
    Wkv = np.asarray(Wkv, np.float32)
    Wout = np.asarray(Wout, np.float32)
    bout = np.asarray(bout, np.float32)

    def swap_heads(W):
        # permute output cols j -> j xor 32 within each 64-block
        c = W.shape[1]
        return np.ascontiguousarray(
            W.reshape(D, c // 64, 2, 32)[:, :, ::-1, :].reshape(D, c)
        )

    def lhsT(W):
        # [D, M] -> [128, 4, M] chunk layout
        return np.ascontiguousarray(
            W.reshape(4, P, W.shape[1]).transpose(1, 0, 2).astype(bfdt)
        )

    Wk = Wkv[:, : KVH * DH]
    Wv = Wkv[:, KVH * DH :]
    wq = lhsT(Wq)
    wqs = lhsT(swap_heads(Wq))
    wk = lhsT(Wk)
    wks = lhsT(swap_heads(Wk))
    wv = lhsT(Wv)
    wo = lhsT(Wout)
    gam = np.ascontiguousarray(gamma.reshape(4, P).T)
    bet = np.ascontiguousarray(beta.reshape(4, P).T)
    bo = np.ascontiguousarray(bout.reshape(4, P).T)
    ones = np.ones((P, 1), np.float32)

    # rotary tables (per half)
    j = np.arange(DH)
    inv_freq = 1.0 / (10000.0 ** ((2.0 * (j % 32)) / DH))
    base = ((2.0 * (j % 32)) + 0.4 * DH) / (1.4 * DH)
    sign = np.where(j < 32, -1.0, 1.0)

    tables = []
    for half in range(2):
        pos = half * NLOC + np.arange(NLOC, dtype=np.float64)
        freqs = pos[None, :] * inv_freq[:, None]  # [64, NLOC]
        cos, sin = np.cos(freqs), np.sin(freqs)
        power = (pos - N // 2) / CONTEXT_LEN
        xsc = base[:, None] ** power[None, :]
        cq = np.tile((cos * xsc), (2, 1)).astype(np.float32)
        sq = np.tile((sign[:, None] * sin * xsc), (2, 1)).astype(np.float32)
        ck = np.tile((cos / xsc), (2, 1)).astype(np.float32)
        sk = np.tile((sign[:, None] * sin / xsc), (2, 1)).astype(np.float32)
        tables.append((cq, sq, ck, sk))

    in_maps = []
    for core in range(NCORES):
        b, half = core // 2, core % 2
        xc = np.ascontiguousarray(
            x[b].reshape(4, P, N)[:, :, half * NLOC : (half + 1) * NLOC]
        )
        cq, sq, ck, sk = tables[half]
        in_maps.append(
            {
                "x": xc, "wq": wq, "wqs": wqs, "wk": wk, "wks": wks,
                "wv": wv, "wo": wo, "cq": cq, "sq": sq, "ck": ck, "sk": sk,
                "gam": gam, "bet": bet, "bo": bo, "ones": ones,
                "onesb": ones.astype(bfdt),
            }
        )
    return in_maps


def kernel(x, gamma, beta, Wq, Wkv, Wout, bout):
    trace = os.environ.get("KERNEL_TRACE", "0") == "1"
    if trace:
        _enable_trace_hook()
    trivial_gb = bool(
        np.all(np.asarray(gamma) == 1.0) and np.all(np.asarray(beta) == 0.0)
    )
    trivial_bo = bool(np.all(np.asarray(bout) == 0.0))
    if "nc" not in _CACHE:
        _CACHE["nc"] = _build(trivial_gb, trivial_bo)
        _CACHE["trivial_gb"] = (trivial_gb, trivial_bo)
    assert _CACHE["trivial_gb"] == (trivial_gb, trivial_bo)
    nc = _CACHE["nc"]
    in_maps = _host_inputs(x, gamma, beta, Wq, Wkv, Wout, bout)
    res = run_bass_kernel_spmd(nc, in_maps, list(range(NCORES)), trace=trace)
    if trace and res.exec_time_ns is not None:
        print(f"HW exec time: {res.exec_time_ns} ns")
        _CACHE["exec_time_ns"] = res.exec_time_ns

    y = np.empty((B, D, N), np.float32)
    for core in range(NCORES):
        b, half = core // 2, core % 2
        y[b, :, half * NLOC : (half + 1) * NLOC] = res.results[core]["out"].reshape(
            D, NLOC
        )
    return y
